# revision 49
# baseline (speedup 1.0000x reference)
"""Trainium2 Bass kernel for batched global mean pooling (segment mean).

Computes, for N sorted nodes with 64 features and G graphs:
    out[g, f] = mean over nodes n with batch[n] == g of node_features[n, f]
(empty graphs -> zeros), distributed over 8 NeuronCores.

Strategy (graph sharding; no collectives; all-fp8 tri-engine streaming):
  - Core k owns 128 graphs. batch is sorted, so each graph's nodes are a
    contiguous row range of node_features. Partition p of core k carries
    local graph p's nodes.
  - The whole stream ships as fp8 E3M4 (1 B/elem). Partials accumulate
    in fp32/fp16, so input rounding dominates the error; averaged over
    ~2000 nodes per graph it lands at ~1.3e-2 max relative error, under
    the 2e-2 gate.
  - The whole per-core stream (~16 MB) is RESIDENT in SBUF -- no buffer
    recycling -- so every chunk DMA is wait-free at issue and the two
    HWDGE rings run at the HBM ceiling (~360-430 B/ns/core under 8-core
    contention). Chunk order is earliest-deadline-first from each
    engine's zero-idle finish plan; ring choice greedily balances bytes.
  - The stream is split across THREE compute engines in proportion to
    their drain rates, because no single engine can match the DMA supply
    (the PE retires exactly one 128-lane column per cycle regardless of
    dtype -- fp8 gives no moving-data speedup):
      * PE stream (~62%, 303 B/ns): chunks in (node, feat) layout; each
        8-node slab is a [128, 512] matmul identity.T @ slab accumulated
        into a ping-pong pair of PSUM banks (partition p = graph p).
      * DVE stream (~24%, ~116 B/ns): chunks packed (feat, node) so
        tensor_reduce sums the contiguous node axis at full DVE rate;
        each chunk's [128, 64] fp16 partial lands in its own slot.
      * Pool stream (~14%, ~64 B/ns): (node, feat) chunks collapsed by a
        pairwise fp8->fp16 add tree into slots.
  - Overflow: graphs larger than the uniform main capacity spill their
    remainder into overflow slots (slot p = a partition-row of PSUM bank
    B holding up to 8*M1 nodes of ONE graph), capping per-partition
    padding near the MEAN graph size. It ships early, so bank B's fold
    runs mid-stream.
  - Tail: the DVE folds PSUM bank A right after the last main matmul;
    the PE preloads fp16 Wm = diag(1/count) once and accumulates
        out_psum = sum_k Wm.T @ slot_k + Wm.T @ fold_A + Wo.T @ fold_B
    into one PSUM group (each slot matmul fires as its reduce lands; Wo
    scatters overflow slots, both carry the mean division). One [128,64]
    DMA out per core; host concatenates.

The Bass program is compiled per call with the split and chunk plans
derived from the actual input, so any node/graph distribution is
handled.
"""

import math

import numpy as np

import concourse.mybir as mybir
import concourse.tile as tile
from concourse import bacc
from concourse.bass_utils import run_bass_kernel_spmd
from concourse.masks import make_identity

NCORES = 8
P = 128  # partitions = local graphs per core
F = 64  # features
B = 8  # nodes per matmul slab: 8*64 = 512 f32 = one full PSUM bank
PE_TB = 192  # nodes per bulk PE DMA chunk (12 KB per partition row at fp8)
DVE_TB = 64  # nodes per bulk DVE DMA chunk (4 KB rows; 128-node chunks were tried
             # for DMA row efficiency but their coarser post-arrival lumpiness
             # lengthened the tail)

# set by tests to capture a profile; harness path leaves these alone
TRACE = False
LAST_RESULTS = None


def _pe_chunks(total):
    """PE-stream chunk plan: small ramp chunks first (fast pipeline start
    while the DMA queues cold-start), then 256-node bulk chunks. The PE
    lags the DMA at the stream end anyway, so no tail shaping. All sizes
    mult of 8."""
    ramp = [8, 16, 32, 64, 128]
    tail = [64, 32]
    if total < sum(ramp) + sum(tail) + PE_TB:
        out = []
        t = 0
        while t < total:
            n = min(64, total - t)
            out.append((t, n))
            t += n
        return out
    mid = total - sum(ramp) - sum(tail)
    nbulk, rem = divmod(mid, PE_TB)
    sizes = ramp + [PE_TB] * nbulk + ([rem] if rem else []) + tail
    out = []
    t = 0
    for n in sizes:
        out.append((t, n))
        t += n
    assert t == total
    return out


def _dve_chunks(total):
    """DVE-stream chunk plan: 64-node bulk chunks with a small final chunk
    so the last reduce on the critical path is ~1 us, not ~5."""
    if total <= 0:
        return []
    sizes = []
    rem = total
    while rem >= DVE_TB + 48:
        sizes.append(DVE_TB)
        rem -= DVE_TB
    # descending tail so the last reduces on the critical path are short
    for sz in (rem - 48, 32, 16) if rem > 48 else (rem - 16, 16):
        if sz > 0:
            sizes.append(sz)
    out = []
    t = 0
    for n in sizes:
        out.append((t, n))
        t += n
    assert t == total
    return out


def _pool_chunks(total):
    """Pool-stream chunk plan: power-of-two sizes (the pairwise add tree
    halves cleanly), 64-node bulk, small power-of-two tail chunks."""
    out = []
    t = 0
    rem = total
    while rem >= 64:
        out.append((t, 64))
        t += 64
        rem -= 64
    for sz in (32, 16, 8):
        if rem >= sz:
            out.append((t, sz))
            t += sz
            rem -= sz
    assert rem == 0 and t == total
    return out


def _build(m_pe, dve_chunks, pool_chunks, m1):
    nc = bacc.Bacc("TRN2", target_bir_lowering=False, debug=False, num_devices=1)
    pe_n = B * m_pe  # PE main nodes per partition
    dve_n = sum(n for _, n in dve_chunks)  # DVE nodes per partition
    pool_n = sum(n for _, n in pool_chunks)  # Pool nodes per partition
    cap1 = B * m1  # overflow nodes per slot
    total_n = pe_n + dve_n + pool_n + cap1
    hl8 = nc.dram_tensor(
        "hl8", [P, total_n * F], mybir.dt.float8e3, kind="ExternalInput"
    ).ap()
    n_w = 2 if m1 else 1
    wm = nc.dram_tensor("wm", [P, n_w * P], mybir.dt.float16, kind="ExternalInput").ap()
    out = nc.dram_tensor("out", [P, F], mybir.dt.float32, kind="ExternalOutput").ap()

    n_mm = m_pe + m1
    nslots = len(dve_chunks) + len(pool_chunks)
    keep_ldw_names = []
    with tile.TileContext(nc) as tc:
        with (
            tc.tile_pool(name="consts", bufs=1) as consts,
            tc.tile_pool(name="stream", bufs=1) as stream,
            tc.tile_pool(name="ep", bufs=1) as ep,
            tc.tile_pool(name="acc", bufs=1, space="PSUM") as accp,
        ):
            # build the fp8 identity on-device (Pool engine) so the first
            # weight preload has no DMA dependency
            ident_sb = consts.tile([P, P], mybir.dt.float8e3)
            make_identity(nc, ident_sb[:])
            ldw_id = nc.tensor.ldweights(ident_sb[:])
            keep_ldw_names.append(ldw_id.ins.name)

            # main stream ping-pongs between TWO PSUM banks (halves of one
            # 1024-wide tile) so consecutive matmuls never hit the same
            # bank back-to-back; overflow gets its own bank
            pp = m_pe >= 2
            psum_a = accp.tile([P, 1024 if pp else 512], mybir.dt.float32)
            psum_b = None
            if m1:
                psum_b = accp.tile([P, 512], mybir.dt.float32, name="psum_b")
            psum_o = accp.tile([P, F], mybir.dt.float32)
            # fp16 partials: the DVE reduce accumulates in fp32
            # internally and only the final per-feature sums round to
            # fp16 (~5e-4 rel, dwarfed by the fp8 input rounding); fp16
            # lets the combine matmuls use preloaded fp16 weights
            slots = ep.tile([P, max(nslots, 1) * F], mybir.dt.float16, name="slots")

            # The whole stream is resident in SBUF (no buffer recycling):
            # every chunk DMA is wait-free at issue, so the two HWDGE rings
            # stay full and run at the HBM ceiling; consumers read slices
            # as chunks land.
            sb_pe = stream.tile([P, max(pe_n + cap1, 1) * F], mybir.dt.float8e3)
            sb_dve = (
                stream.tile([P, dve_n * F], mybir.dt.float8e3, name="sb_dve")
                if dve_n
                else None
            )
            sb_pool = (
                stream.tile([P, pool_n * F], mybir.dt.float8e3, name="sb_pool")
                if pool_n
                else None
            )
            # pairwise-add tree scratch for the Pool stream (fp16 partials)
            sc = (
                ep.tile([P, 32 * F], mybir.dt.float16, name="sc") if pool_n else None
            )

            # merged DMA issue order, paced by consumer drain rate: always
            # issue for the stream whose engine is closest to running dry
            # (bytes issued / consumption rate, B/ns: PE ~303, DVE ~116,
            # Pool ~70). Starving the PE early was worth ~10 us of idle.
            pe_seq = [("pe", t0, nt) for t0, nt in _pe_chunks(pe_n)]
            if m1:
                # overflow ships EARLY: PSUM bank B then closes mid-stream
                # and its fold runs on the DVE long before the tail
                pe_seq.insert(min(5, len(pe_seq)), ("ovf", 0, cap1))
            # Earliest-deadline-first issue order. Chunk deadlines come
            # from each engine's zero-idle finish plan worked BACKWARD at
            # its drain rate (B/ns: PE 303, DVE 116, Pool 64): chunk k of
            # stream i must arrive by finish_i - (bytes_after_k / d_i).
            # Finishes are staggered: the DVE ends 3 us early (the fold +
            # slotfold chain runs on it before the combine), Pool 1.5 us
            # early, the PE last (its post-arrival lag is one small tail
            # chunk). The common supply rate shifts all deadlines equally,
            # so the ORDER needs no supply estimate. This front-loads the
            # PE (largest bytes/drain ratio) without leaving it an end
            # backlog the way plain byte-proportional pacing did.
            streams = [
                (pe_seq, 303.0, 0.0),
                ([("dve", t0, nt) for t0, nt in dve_chunks], 116.0, 2000.0),
                ([("pool", t0, nt) for t0, nt in pool_chunks], 64.0, 1500.0),
            ]
            order = []
            for seq, d_rate, stagger in streams:
                btot = sum(c[2] for c in seq) * F * P
                cum = 0
                for c in seq:
                    cum += c[2] * F * P
                    deadline = -stagger - (btot - cum) / d_rate
                    order.append((deadline, len(order), c))
            order.sort()
            issue = [c for _, _, c in order]
            issue.insert(min(4, len(issue)), ("wm", 0, 0))

            wm_sb = consts.tile([P, n_w * P], mybir.dt.float16)

            # NOTE: row-splitting each transfer across both rings was tried
            # to kill cross-stream head-of-line skew; 64-row transfers
            # collapsed per-queue burst length and cost ~25% aggregate DMA
            # bandwidth. Full-width transfers with greedy per-ring BYTE
            # balancing win: each ring's local queue then tracks the global
            # pacing order, so chunks arrive roughly when the pacer
            # intended instead of piling up behind one ring's big chunks.
            ring_bytes = [0, 0]

            def dma2(dst, src):
                ri = 0 if ring_bytes[0] <= ring_bytes[1] else 1
                ring_bytes[ri] += src.free_size()
                (nc.sync if ri == 0 else nc.scalar).dma_start(dst, src)

            mm = 0
            dve_i = 0
            first_mm = True
            prev_mm_inst = None
            reduces = []
            sm = ep.tile([P, F], mybir.dt.float16)
            so = ep.tile([P, F], mybir.dt.float16, name="so") if m1 else None
            for kind, t0, nt in issue:
                if kind == "wm":
                    nc.sync.dma_start(wm_sb[:], wm[:])
                    continue
                if kind == "pe" or kind == "ovf":
                    # pe region: [0, pe_n); ovf region right after it
                    loc = t0 if kind == "pe" else pe_n + t0
                    base = t0 if kind == "pe" else pe_n + dve_n + pool_n + t0
                    dma2(
                        sb_pe[:, loc * F : (loc + nt) * F],
                        hl8[:, base * F : (base + nt) * F],
                    )
                    # PSUM roles (bank, start/stop) are derived from the
                    # chunk's STREAM position t0, not DMA issue order: the
                    # overflow chunk is issued out of order
                    for bB in range(nt // B):
                        idx = t0 // B + bB
                        if kind == "pe":
                            half = (idx & 1) if pp else 0
                            reg = psum_a[:, half * 512 : half * 512 + B * F]
                            first = idx < 2 if pp else idx == 0
                            last = idx >= m_pe - 2 if pp else idx == m_pe - 1
                        else:
                            reg = psum_b[:, : B * F]
                            first = idx == 0
                            last = idx == m1 - 1
                        inst = nc.tensor.matmul(
                            reg,
                            ident_sb[:],
                            sb_pe[:, (loc + bB * B) * F : (loc + (bB + 1) * B) * F],
                            start=first,
                            stop=last,
                        )
                        inst.ins.ldweights = False
                        if first_mm:
                            tile.add_dep_helper(
                                inst.ins,
                                ldw_id.ins,
                                sync=False,
                                reason="identity preloaded once",
                            )
                            first_mm = False
                        prev_mm_inst = inst
                        mm += 1
                    if kind == "ovf":
                        # bank B is closed; fold it now, mid-stream
                        with nc.allow_low_precision(reason="fp16 partials"):
                            nc.vector.tensor_reduce(
                                so[:],
                                psum_b[:, 0 : B * F].rearrange(
                                    "p (b f) -> p f b", b=B
                                ),
                                axis=mybir.AxisListType.X,
                                op=mybir.AluOpType.add,
                            )
                elif kind == "dve":  # (feat, node) layout, contiguous node axis
                    base = pe_n + t0
                    dma2(
                        sb_dve[:, t0 * F : (t0 + nt) * F],
                        hl8[:, base * F : (base + nt) * F],
                    )
                    with nc.allow_low_precision(reason="fp16 partials"):
                        red = nc.vector.tensor_reduce(
                            slots[:, dve_i * F : (dve_i + 1) * F],
                            sb_dve[:, t0 * F : (t0 + nt) * F].rearrange(
                                "p (f n) -> p f n", f=F
                            ),
                            axis=mybir.AxisListType.X,
                            op=mybir.AluOpType.add,
                        )
                    reduces.append(red)
                    dve_i += 1
                else:  # pool chunk: (node, feat) layout, pairwise add tree
                    base = pe_n + dve_n + t0
                    dma2(
                        sb_pool[:, t0 * F : (t0 + nt) * F],
                        hl8[:, base * F : (base + nt) * F],
                    )
                    src = sb_pool[:, t0 * F : (t0 + nt) * F]
                    h = nt // 2
                    if h == 1:  # 2-node chunk straight into its slot
                        nc.gpsimd.tensor_tensor(
                            slots[:, dve_i * F : (dve_i + 1) * F],
                            src[:, :F],
                            src[:, F : 2 * F],
                            op=mybir.AluOpType.add,
                        )
                    else:
                        nc.gpsimd.tensor_tensor(
                            sc[:, : h * F],
                            src[:, : h * F],
                            src[:, h * F : nt * F],
                            op=mybir.AluOpType.add,
                        )
                        while h > 2:
                            nc.gpsimd.tensor_tensor(
                                sc[:, : (h // 2) * F],
                                sc[:, : (h // 2) * F],
                                sc[:, (h // 2) * F : h * F],
                                op=mybir.AluOpType.add,
                            )
                            h //= 2
                        nc.gpsimd.tensor_tensor(
                            slots[:, dve_i * F : (dve_i + 1) * F],
                            sc[:, :F],
                            sc[:, F : 2 * F],
                            op=mybir.AluOpType.add,
                        )
                    dve_i += 1
            assert mm == n_mm and dve_i == nslots

            # fold PSUM bank A (DVE, strided reduce; needs the main
            # accumulation group closed, so it lands right after the last
            # main matmul)
            nb = 2 * B if pp else B
            with nc.allow_low_precision(reason="fp16 partials"):
                nc.vector.tensor_reduce(
                    sm[:],
                    psum_a[:, 0 : nb * F].rearrange("p (b f) -> p f b", b=nb),
                    axis=mybir.AxisListType.X,
                    op=mybir.AluOpType.add,
                )

            # combine + mean-divide in one PSUM accumulation group on the
            # PE. Weights are fp16 (1/count fits easily), so Wm can be
            # preloaded ONCE and each DVE/Pool slot becomes its own 64-col
            # matmul that fires as soon as that slot's reduce lands -- no
            # serial slotfold on the DVE tail.
            # the overflow term goes FIRST: so was folded mid-stream, so
            # the Wo matmul + its ldweights run before fold_A even exists,
            # and the post-fold_A critical path is just one matmul
            prev_combine = None
            if m1:
                ldw_wo = nc.tensor.ldweights(wm_sb[:, P : 2 * P])
                keep_ldw_names.append(ldw_wo.ins.name)
                if prev_mm_inst is not None:
                    tile.add_dep_helper(
                        ldw_wo.ins,
                        prev_mm_inst.ins,
                        sync=False,
                        reason="Wo loads after the last identity matmul",
                    )
                cf = nc.tensor.matmul(
                    psum_o[:], wm_sb[:, P : 2 * P], so[:], start=True, stop=False
                )
                cf.ins.ldweights = False
                tile.add_dep_helper(
                    cf.ins, ldw_wo.ins, sync=False, reason="Wo preloaded once"
                )
                prev_combine = cf
            ldw_wm = nc.tensor.ldweights(wm_sb[:, 0:P])
            keep_ldw_names.append(ldw_wm.ins.name)
            tile.add_dep_helper(
                ldw_wm.ins,
                (prev_combine or prev_mm_inst).ins,
                sync=False,
                reason="Wm loads after the Wo matmul / last identity matmul",
            )
            comb = []
            movings = [
                slots[:, k * F : (k + 1) * F] for k in range(nslots)
            ] + [sm[:]]
            for j, mov in enumerate(movings):
                comb.append(
                    nc.tensor.matmul(
                        psum_o[:],
                        wm_sb[:, 0:P],
                        mov,
                        start=(not m1) and j == 0,
                        stop=j == len(movings) - 1,
                    )
                )
            for c in comb:
                c.ins.ldweights = False
            tile.add_dep_helper(
                comb[0].ins, ldw_wm.ins, sync=False, reason="Wm preloaded once"
            )
            res = ep.tile([P, F], mybir.dt.float32)
            nc.scalar.activation(res[:], psum_o[:], mybir.ActivationFunctionType.Copy)
            # split the result DMA across both rings: the two 16 KB halves
            # issue and fly in parallel, halving the end-of-kernel latency
            nc.sync.dma_start(out[0 : P // 2], res[0 : P // 2])
            nc.scalar.dma_start(out[P // 2 : P], res[P // 2 : P])

    nc.compile()
    # bacc can materialize one Ldweights per Matmult even with
    # ldweights=False; the streaming matmuls rely on the explicit preloads
    # above. Drop every other identity reload that carries no semaphore
    # waits/updates; the explicit preloads are kept by name.
    keep_names = set(keep_ldw_names)
    for fn in nc.m.functions:
        for blk in fn.blocks:
            keep = [
                inst
                for inst in blk.instructions
                if not (
                    isinstance(inst, mybir.InstLdweights)
                    and inst.name not in keep_names
                    and (
                        inst.sync_info is None
                        or (
                            len(inst.sync_info.on_wait) == 0
                            and len(inst.sync_info.on_update) == 0
                        )
                    )
                )
            ]
            if len(keep) != len(blk.instructions):
                blk.instructions = keep
    # NOTE: a pass that hoisted the first 8 chunk DMAs ahead of the Tile
    # preamble barrier (saving ~0.8 us of DMA queue cold-start) was
    # removed: combined with the SBUF-resident stream and profiling-
    # induced timing jitter it produced intermittent corrupted outputs
    # (~1 in 4 traced runs) -- early DMA completions racing the preamble's
    # semaphore state.
    # Trim the TileContext epilogue: after the first all-engine barrier
    # (which guarantees every engine and DMA queue is quiescent and the
    # output is in DRAM), the remaining semaphore RANGE_CLEAR + second
    # barrier are redundant -- the NEFF's own per-engine epilogue zeroes
    # the entire semaphore space anyway.
    for fn in nc.m.functions:
        for blk in fn.blocks:
            if not blk.name.endswith("_end"):
                continue
            isa_idx = [
                i
                for i, inst in enumerate(blk.instructions)
                if isinstance(inst, mybir.InstISA)
            ]
            if isa_idx:
                cut = isa_idx[0]
                if cut > 0 and isinstance(blk.instructions[cut - 1], mybir.InstDrain):
                    cut -= 1
                blk.instructions = blk.instructions[:cut]
    return nc


def _plan(counts, gpc):
    """Pick (M0, M1): per-partition main/overflow capacities (in 8-node
    units) minimizing stream length s.t. every core's overflow fits in 128
    slots of 8*M1 nodes. counts is laid out [NCORES * gpc]."""
    t_max = int(counts.max()) if counts.size else 1
    s_max = math.ceil(t_max / B)
    percore = counts.reshape(NCORES, gpc)
    best = (s_max, s_max, 0)  # no-overflow fallback
    for m0 in range(1, s_max):
        ovf = np.maximum(percore - B * m0, 0)
        for m1 in range(1, s_max - m0):
            if m0 + m1 >= best[0]:
                break
            slots = np.ceil(ovf / (B * m1)).sum(axis=1).max()
            if slots <= P:
                best = (m0 + m1, m0, m1)
                break
    return best[1], best[2]


def kernel(node_features, batch, num_graphs):
    global LAST_RESULTS
    x = np.asarray(node_features, dtype=np.float32)
    b = np.asarray(batch, dtype=np.int64).ravel()
    G = int(num_graphs)
    N = x.shape[0]
    assert x.shape[1] == F, f"expected {F} features, got {x.shape[1]}"

    if not np.all(b[1:] >= b[:-1]):  # defensive: layout relies on sorted batch
        order = np.argsort(b, kind="stable")
        b = b[order]
        x = x[order]

    gpc = math.ceil(G / NCORES)  # local graphs per core
    assert gpc <= P, f"num_graphs {G} too large for {NCORES} cores x {P} partitions"

    # ids >= G (if any) are dropped, matching segment_sum(num_segments=G)
    counts = np.bincount(b, minlength=NCORES * gpc)[: NCORES * gpc].astype(np.int64)
    starts = np.zeros(NCORES * gpc + 1, dtype=np.int64)
    np.cumsum(counts, out=starts[1:])
    m0, m1 = _plan(counts, gpc)
    # split the main capacity between the PE (identity matmuls, ~303
    # nodes/us at fp8) and the DVE (contiguous-axis tensor_reduce, ~99
    # nodes/us) so their combined ingest matches the DMA ceiling
    # split the main stream across three consumers in proportion to their
    # drain rates (ns per 8-node unit: PE 216, DVE ~552, Pool ~930).
    # Weighting by the PE's EFFECTIVE rate (~260 under DMA contention)
    # was tried and regressed: the extra work pushed onto the DVE/Pool
    # extends their serial post-arrival tails more than it relieves the
    # PE's end-of-stream backlog.
    wsum = 1 / 216 + 1 / 552 + 1 / 930
    m_dve = round(m0 * (1 / 552) / wsum)
    m_pool = round(m0 * (1 / 930) / wsum)
    m_pe = m0 - m_dve - m_pool
    pe_n = B * m_pe
    dve_n = B * m_dve
    pool_n = B * m_pool
    cap0 = B * m0  # total main nodes per partition
    cap1 = B * m1  # overflow nodes per slot
    dve_chunks = _dve_chunks(dve_n)
    pool_chunks = _pool_chunks(pool_n)

    x_ext = np.vstack([x, np.zeros((1, F), dtype=np.float32)])  # row N = zeros
    col0 = np.arange(cap0, dtype=np.int64)

    np8 = mybir.dt.np(mybir.dt.float8e3)
    in_maps = []
    for k in range(NCORES):
        g0 = k * gpc
        cg = counts[g0 : g0 + gpc]
        sg = starts[g0 : g0 + gpc]
        inv = np.where(cg > 0, 1.0 / np.maximum(cg, 1), 0.0).astype(np.float32)

        cmain = np.minimum(cg, cap0)
        idx = np.where(col0[None, :] < cmain[:, None], sg[:, None] + col0[None, :], N)
        if gpc < P:  # pad partitions when graph count is not divisible by 8
            idx = np.vstack([idx, np.full((P - gpc, cap0), N, dtype=np.int64)])

        n_w = 2 if m1 else 1
        w = np.zeros((P, n_w * P), dtype=np.float32)
        w[np.arange(gpc), np.arange(gpc)] = inv

        if m1:
            # assign overflow slots: consecutive 8*m1-node pieces of each
            # overflow graph's tail, packed into partition-rows of bank B
            oidx = np.full((P, cap1), N, dtype=np.int64)
            slot = 0
            for g in range(gpc):
                ovf = int(cg[g] - cap0)
                pos = int(sg[g] + cap0)
                while ovf > 0:
                    take = min(ovf, cap1)
                    assert slot < P, "overflow slots exhausted (planner bug)"
                    oidx[slot, :take] = pos + np.arange(take)
                    w[slot, P + g] = inv[g]
                    pos += take
                    ovf -= take
                    slot += 1
            idx = np.hstack([idx, oidx])

        feats = x_ext[idx].astype(np8)  # [P, cap0(+cap1), F] fp8
        parts = [feats[:, :pe_n].reshape(P, -1)]
        for t0, nt in dve_chunks:  # (feat, node) per chunk for the DVE
            parts.append(
                np.ascontiguousarray(
                    feats[:, pe_n + t0 : pe_n + t0 + nt].transpose(0, 2, 1)
                ).reshape(P, -1)
            )
        # pool region keeps (node, feat) layout
        parts.append(feats[:, pe_n + dve_n : cap0].reshape(P, -1))
        if m1:
            parts.append(feats[:, cap0:].reshape(P, -1))
        in_maps.append(
            {"hl8": np.concatenate(parts, axis=1), "wm": w.astype(np.float16)}
        )

    nc = _build(m_pe, dve_chunks, pool_chunks, m1)
    try:
        res = run_bass_kernel_spmd(
            nc, in_maps, core_ids=list(range(NCORES)), trace=TRACE
        )
    except Exception:
        # transient device state (e.g. a previous run left a core wedged)
        # has been observed to clear on retry
        res = run_bass_kernel_spmd(
            nc, in_maps, core_ids=list(range(NCORES)), trace=TRACE
        )
    LAST_RESULTS = res

    out = np.concatenate([res.results[k]["out"] for k in range(NCORES)], axis=0)
    return out[:G]
